# revision 4
# baseline (speedup 1.0000x reference)
"""Trainium2 Bass kernel for nn_AttentionAggregator (segment_reduce).

Math: out[b, g] = sum_{j in group g} softmax_g(att)[j] * feat[b, flat_idx[j]]
    = (feat @ W)[b, g]   with W[k, g] = sum_{j in g, flat_idx[j] = k} attn[j]

The segment softmax and the scatter that builds W involve only the tiny
index/weight tensors, so they run on host.  The heavy part — the
(4096 x 4096) @ (4096 x 1024) product — runs on 8 NeuronCores with the
batch axis sharded 512 rows per core (embarrassingly parallel, no
collectives).  Each core computes outT = W^T-blocks x featT via 256
accumulating fp32r matmuls (lhsT = W k/g-tile, rhs = featT k-tile).

Self-contained: hardcodes shapes from the problem spec; no sibling imports.
"""

import numpy as np

B = 4096
NG = 4096
G = 1024
N_CORES = 8
B_LOC = B // N_CORES          # 512 batch rows per core
P = 128                       # SBUF/PE partitions
KT = NG // P                  # 32 contraction tiles
GT = G // P                   # 8 output-group tiles

_NC_CACHE = {}


def _host_softmax_scatter(att_weights, flat_idx, segment_ids, num_segments):
    """Per-segment softmax of att_weights, scatter-added into dense W (NG, G)."""
    aw = np.asarray(att_weights, dtype=np.float32)
    seg = np.asarray(segment_ids, dtype=np.int64)
    idx = np.asarray(flat_idx, dtype=np.int64)
    n_seg = int(num_segments)

    seg_max = np.full(n_seg, -np.inf, dtype=np.float32)
    np.maximum.at(seg_max, seg, aw)
    ex = np.exp(aw - seg_max[seg])
    denom = np.zeros(n_seg, dtype=np.float32)
    np.add.at(denom, seg, ex)
    attn = ex / denom[seg]

    w = np.zeros((NG, n_seg), dtype=np.float32)
    np.add.at(w, (idx, seg), attn)
    return w


def build_nc():
    """SPMD single-core program: outT(1024, 512) = W(4096, 1024)^T @ featT(4096, 512)."""
    import concourse.mybir as mybir
    from concourse import bacc
    from concourse.tile import TileContext

    F32 = mybir.dt.float32
    F32R = mybir.dt.float32r

    nc = bacc.Bacc("TRN2", target_bir_lowering=False, debug=False)
    # Host pre-tiles both operands into the exact SBUF layouts so every DMA
    # is fully contiguous per partition (64KB/16KB runs, not 512B).
    # feat_t[p, k, b] = feat[core_b0 + b, k*128 + p]
    feat_t = nc.dram_tensor("featT", [P, KT, B_LOC], F32R, kind="ExternalInput")
    # wmat[gt, p, k, g] = W[k*128 + p, gt*128 + g]
    wmat = nc.dram_tensor("wmat", [GT, P, KT, P], F32R, kind="ExternalInput")
    out_t = nc.dram_tensor("outT", [G, B_LOC], F32, kind="ExternalOutput")

    FEAT_CHUNK = 8                      # k-tiles per feat DMA (2 MB each)

    with TileContext(nc) as tc:
        with (
            tc.tile_pool(name="featp", bufs=1) as featp,
            tc.tile_pool(name="wp", bufs=3) as wp,
            tc.tile_pool(name="pp", bufs=2, space="PSUM") as pp,
            tc.tile_pool(name="op", bufs=2) as op,
        ):
            feat_sb = featp.tile([P, KT, B_LOC], F32R)
            for c in range(0, KT, FEAT_CHUNK):
                nc.sync.dma_start(
                    feat_sb[:, c : c + FEAT_CHUNK, :],
                    feat_t[:, c : c + FEAT_CHUNK, :],
                )
            for gt in range(GT):
                w_sb = wp.tile([P, KT, P], F32R, tag="w")
                nc.sync.dma_start(w_sb, wmat[gt])
                ps = pp.tile([P, B_LOC], mybir.dt.float32, tag="ps")
                for k in range(KT):
                    nc.tensor.matmul(
                        ps,
                        lhsT=w_sb[:, k, :],
                        rhs=feat_sb[:, k, :],
                        start=(k == 0),
                        stop=(k == KT - 1),
                    )
                o_sb = op.tile([P, B_LOC], F32, tag="o")
                nc.scalar.copy(o_sb, ps)
                nc.sync.dma_start(out_t[gt * P : (gt + 1) * P, :], o_sb)
    nc.compile()
    return nc


def make_in_maps(gene_set_features, wmat):
    feat = np.asarray(gene_set_features, dtype=np.float32)  # (B, NG)
    # (GT, P, KT, P): wmat_tiled[gt, p, k, g] = W[k*128 + p, gt*128 + g]
    w_tiled = np.ascontiguousarray(
        wmat.reshape(KT, P, GT, P).transpose(2, 1, 0, 3)
    )
    in_maps = []
    for c in range(N_CORES):
        shard = feat[c * B_LOC : (c + 1) * B_LOC, :]  # (B_LOC, NG)
        # (P, KT, B_LOC): feat_tiled[p, k, b] = shard[b, k*128 + p]
        feat_tiled = np.ascontiguousarray(
            shard.T.reshape(KT, P, B_LOC).transpose(1, 0, 2)
        )
        in_maps.append({"featT": feat_tiled, "wmat": w_tiled})
    return in_maps


def kernel(gene_set_features, att_weights, flat_idx, segment_ids, num_segments):
    from concourse.bass_utils import run_bass_kernel_spmd

    wmat = _host_softmax_scatter(att_weights, flat_idx, segment_ids, num_segments)
    in_maps = make_in_maps(gene_set_features, wmat)

    if "nc" not in _NC_CACHE:
        _NC_CACHE["nc"] = build_nc()
    nc = _NC_CACHE["nc"]

    res = run_bass_kernel_spmd(nc, in_maps, core_ids=list(range(N_CORES)))

    out = np.empty((B, G), dtype=np.float32)
    for c in range(N_CORES):
        out[c * B_LOC : (c + 1) * B_LOC, :] = res.results[c]["outT"].T
    return out


# revision 6
# speedup vs baseline: 1.2971x; 1.2971x over previous
"""Trainium2 Bass kernel for nn_AttentionAggregator (segment_reduce).

Math: out[b, g] = sum_{j in group g} softmax_g(att)[j] * feat[b, flat_idx[j]]
    = (feat @ W)[b, g]   with W[k, g] = sum_{j in g, flat_idx[j] = k} attn[j]

The segment softmax and the scatter that builds W involve only the tiny
index/weight tensors, so they run on host.  The heavy part — the
(4096 x 4096) @ (4096 x 1024) product — runs on 8 NeuronCores with the
batch axis sharded 512 rows per core (embarrassingly parallel, no
collectives).  Each core computes outT = W^T-blocks x featT via 256
accumulating fp32r matmuls (lhsT = W k/g-tile, rhs = featT k-tile).

Self-contained: hardcodes shapes from the problem spec; no sibling imports.
"""

import numpy as np

B = 4096
NG = 4096
G = 1024
N_CORES = 8
B_LOC = B // N_CORES          # 512 batch rows per core
P = 128                       # SBUF/PE partitions
KT = NG // P                  # 32 contraction tiles
GT = G // P                   # 8 output-group tiles

_NC_CACHE = {}


def _host_softmax_scatter(att_weights, flat_idx, segment_ids, num_segments):
    """Per-segment softmax of att_weights, scatter-added into dense W (NG, G)."""
    aw = np.asarray(att_weights, dtype=np.float32)
    seg = np.asarray(segment_ids, dtype=np.int64)
    idx = np.asarray(flat_idx, dtype=np.int64)
    n_seg = int(num_segments)

    seg_max = np.full(n_seg, -np.inf, dtype=np.float32)
    np.maximum.at(seg_max, seg, aw)
    ex = np.exp(aw - seg_max[seg])
    denom = np.zeros(n_seg, dtype=np.float32)
    np.add.at(denom, seg, ex)
    attn = ex / denom[seg]

    w = np.zeros((NG, n_seg), dtype=np.float32)
    np.add.at(w, (idx, seg), attn)
    return w


def build_nc():
    """SPMD single-core program: outT(1024, 512) = W(4096, 1024)^T @ featT(4096, 512)."""
    import concourse.mybir as mybir
    from concourse import bacc
    from concourse.tile import TileContext

    F32 = mybir.dt.float32
    F16 = mybir.dt.float16

    nc = bacc.Bacc("TRN2", target_bir_lowering=False, debug=False)
    # Host pre-tiles both operands (fp16) into the exact SBUF layouts so
    # every DMA is fully contiguous per partition.
    # feat_t[p, k, b] = feat[core_b0 + b, k*128 + p]
    feat_t = nc.dram_tensor("featT", [P, KT, B_LOC], F16, kind="ExternalInput")
    # wmat[gt, p, k, g] = W[k*128 + p, gt*128 + g]
    wmat = nc.dram_tensor("wmat", [GT, P, KT, P], F16, kind="ExternalInput")
    out_t = nc.dram_tensor("outT", [G, B_LOC], F32, kind="ExternalOutput")

    FEAT_CHUNK = 8                      # k-tiles per feat DMA (1 MB each)

    with TileContext(nc) as tc:
        with (
            tc.tile_pool(name="featp", bufs=1) as featp,
            tc.tile_pool(name="wp", bufs=3) as wp,
            tc.tile_pool(name="pp", bufs=2, space="PSUM") as pp,
            tc.tile_pool(name="op", bufs=2) as op,
        ):
            # feat chunks ride the Scalar HWDGE ring, W stream rides the
            # Sync ring: the first matmul waits only on w[0] + feat chunk 0
            # rather than on one serialized FIFO of everything.
            feat_sb = featp.tile([P, KT, B_LOC], F16)
            for c in range(0, KT, FEAT_CHUNK):
                nc.scalar.dma_start(
                    feat_sb[:, c : c + FEAT_CHUNK, :],
                    feat_t[:, c : c + FEAT_CHUNK, :],
                )
            for gt in range(GT):
                w_sb = wp.tile([P, KT, P], F16, tag="w")
                nc.sync.dma_start(w_sb, wmat[gt])
                ps = pp.tile([P, B_LOC], mybir.dt.float32, tag="ps")
                for k in range(KT):
                    nc.tensor.matmul(
                        ps,
                        lhsT=w_sb[:, k, :],
                        rhs=feat_sb[:, k, :],
                        start=(k == 0),
                        stop=(k == KT - 1),
                    )
                o_sb = op.tile([P, B_LOC], F32, tag="o")
                nc.scalar.copy(o_sb, ps)
                nc.scalar.dma_start(out_t[gt * P : (gt + 1) * P, :], o_sb)
    nc.compile()
    return nc


def make_in_maps(gene_set_features, wmat):
    feat = np.asarray(gene_set_features, dtype=np.float32).astype(np.float16)
    # (GT, P, KT, P): wmat_tiled[gt, p, k, g] = W[k*128 + p, gt*128 + g]
    w_tiled = np.ascontiguousarray(
        wmat.astype(np.float16).reshape(KT, P, GT, P).transpose(2, 1, 0, 3)
    )
    in_maps = []
    for c in range(N_CORES):
        shard = feat[c * B_LOC : (c + 1) * B_LOC, :]  # (B_LOC, NG)
        # (P, KT, B_LOC): feat_tiled[p, k, b] = shard[b, k*128 + p]
        feat_tiled = np.ascontiguousarray(
            shard.T.reshape(KT, P, B_LOC).transpose(1, 0, 2)
        )
        in_maps.append({"featT": feat_tiled, "wmat": w_tiled})
    return in_maps


def kernel(gene_set_features, att_weights, flat_idx, segment_ids, num_segments):
    from concourse.bass_utils import run_bass_kernel_spmd

    wmat = _host_softmax_scatter(att_weights, flat_idx, segment_ids, num_segments)
    in_maps = make_in_maps(gene_set_features, wmat)

    if "nc" not in _NC_CACHE:
        _NC_CACHE["nc"] = build_nc()
    nc = _NC_CACHE["nc"]

    res = run_bass_kernel_spmd(nc, in_maps, core_ids=list(range(N_CORES)))

    out = np.empty((B, G), dtype=np.float32)
    for c in range(N_CORES):
        out[c * B_LOC : (c + 1) * B_LOC, :] = res.results[c]["outT"].T
    return out
